# revision 17
# baseline (speedup 1.0000x reference)
"""Distributed Trainium2 Bass kernel for the ACMProxy loss.

Sharding: proxy bank (N=65536) split across 8 NeuronCores, camera-grouped and
evenly dealt so all cores share one SPMD graph. Each core runs the heavy
(B=64)x(8192) sims matmul (bf16 x stationary, fp8 proxies moving, f32 PSUM)
with the per-row class-match mask folded into PSUM via a 17th accumulation
matmul (identity weights x fp8 mask plane). The only epilogue is a top-8
(InstMax) per (camera x jtile) piece, read straight from PSUM.

Host does everything exact and tiny: positives / class-matching columns
(~7.6k of 65536) and the few overflow proxies are rescored in f32 numpy;
device piece top-8s are merged for hard-negative top-50 (with a per-piece
saturation guard + full-numpy fallback) and per-camera top-5; batch-level
MMD/triplet terms are computed in numpy.

Layout: per (core, half) exactly W=4096 proxy columns (3 cams contiguous,
global slot sizes), jtiles 7x512 + 2x256. DMA: proxP repacked so each
(jtile, k-quarter) is one contiguous 0.5MB chunk in consumption order
(k-major, halves interleaved per quarter); issued on the Sync HWDGE queue.
xm/mask/identity + output flushes ride the Scalar HWDGE queue. The final
quarter is split per-half so almost no matmul work is exposed after the
last streamed byte.
"""

import ml_dtypes
import numpy as np

import concourse.mybir as mybir
from concourse import bacc
from concourse.tile import TileContext
from concourse.bass_utils import run_bass_kernel_spmd

# problem constants (hardcoded)
B, D, N = 64, 2048, 65536
M = 8
KT = D // 128
TEMP = 0.07
NUM_HARDS = 50
LAM_DIS = 0.05
LAM_INS = 0.05
GAMMA = 0.9
NK = 5
MAX_CAMS = 8
NCAMS = 6

W = 4096                      # columns per (core, half)
JTS = [512] * 7 + [256, 256]  # jtile widths (sum == W)
NJT = len(JTS)
HALVES = ((0, 1, 2), (3, 4, 5))
MASK = -224.0                 # fp8e4m3-exact exclusion value; real d in [-6, 6]

_cache = {}


# ---------------------------------------------------------------- layout plan
def _plan(cids):
    """Global cam slots (same for every core) + per-core dealt indices."""
    idx_by_cam = [np.nonzero(cids == c)[0] for c in range(NCAMS)]
    dealt = [[idx_by_cam[c][m::M] for c in range(NCAMS)] for m in range(M)]
    slot = {}
    for camlist in HALVES:
        base = [len(idx_by_cam[c]) // M for c in camlist]
        total = sum(base)
        i = 0
        while total < W:
            base[i % 3] += 1
            total += 1
            i += 1
        while total > W:
            j = int(np.argmax(base))
            base[j] -= 1
            total -= 1
        for c, s in zip(camlist, base):
            slot[c] = s
    offs = {}
    for camlist in HALVES:
        o = 0
        for c in camlist:
            offs[c] = o
            o += slot[c]
    return idx_by_cam, dealt, slot, offs


def _layout(slot, offs):
    """Per-jtile pieces: (cam, half, lo, hi, out_col, wout). wout=8 -> top8,
    wout<8 -> raw copy of the (narrow) intersection."""
    lay = []
    col = 0
    jo = 0
    jstart = []
    for w in JTS:
        jstart.append(col)
        pieces = []
        for h in range(2):
            for c in HALVES[h]:
                lo = max(offs[c], jo)
                hi = min(offs[c] + slot[c], jo + w)
                if lo < hi:
                    wout = 8 if hi - lo >= 8 else hi - lo
                    pieces.append((c, h, lo, hi, col, wout))
                    col += wout
        lay.append((jo, w, pieces))
        jo += w
    # flush [0, bulk) after j6's pieces, [bulk, bulk2) after j7's, rest at end
    return lay, col, (jstart[7], jstart[8])


def _prep_core(m, dealt, slot, offs, proxy, targets, pids):
    """Per-core device inputs: repacked fp8 proxies + fp8 mask plane."""
    col_g = np.full((2, W), -1, dtype=np.int64)
    for c in range(NCAMS):
        h = 0 if c < 3 else 1
        g = dealt[m][c][:slot[c]]
        col_g[h, offs[c]:offs[c] + len(g)] = g
    real = col_g >= 0

    proxT = np.zeros((D, 2, W), dtype=np.float32)
    proxT[:, real] = proxy[col_g[real], :].T

    pid_col = np.where(real, pids[np.where(real, col_g, 0)], -1)  # (2, W)
    # mask plane rows 64h+i: 0 where the column is a negative for batch row i
    # (valid and class mismatch), MASK elsewhere (pad or class match).
    plane = np.empty((128, W), dtype=np.float32)
    for h in range(2):
        neg = real[h][None, :] & (targets[:, None] != pid_col[h][None, :])
        plane[64 * h:64 * h + 64] = np.where(neg, 0.0, MASK)

    # repack fp8, per jtile: [q(4)][h(2)][kk(4)][w] (consumption order)
    A = proxT.astype(ml_dtypes.float8_e4m3).reshape(KT, 128, 2, W)
    parts = []
    jo = 0
    for w in JTS:
        blk = A[:, :, :, jo:jo + w]                # (KT,128,2,w)
        b = blk.reshape(4, 4, 128, 2, w)           # (q,kk,128,h,w)
        b = np.transpose(b, (2, 0, 3, 1, 4))       # (128,q,h,kk,w)
        parts.append(b.reshape(128, 2 * KT * w))
        jo += w
    proxP = np.ascontiguousarray(np.concatenate(parts, axis=1))
    return {
        "proxP": proxP,
        "pm": np.ascontiguousarray(plane.astype(ml_dtypes.float8_e4m3)),
    }


# ---------------------------------------------------------------- bass kernel
def _build(slot, offs):
    f32 = mybir.dt.float32
    bf16 = mybir.dt.bfloat16
    fp8 = mybir.dt.float8e4
    lay, ncols, (bulk, bulk2) = _layout(slot, offs)
    nc = bacc.Bacc("TRN2", target_bir_lowering=False, debug=False, num_devices=M)

    proxP_e = nc.dram_tensor("proxP", [128, 2 * KT * W], fp8, kind="ExternalInput").ap()
    xm_e = nc.dram_tensor("xm", [128, KT * B], bf16, kind="ExternalInput").ap()
    pm_e = nc.dram_tensor("pm", [128, W], fp8, kind="ExternalInput").ap()
    o_all = nc.dram_tensor("o_all", [128, ncols], f32, kind="ExternalOutput").ap()

    with TileContext(nc) as tc:
        with (
            tc.tile_pool(name="const", bufs=1) as constp,
            tc.tile_pool(name="scr", bufs=3) as scrp,
            tc.tile_pool(name="ps", bufs=6, space="PSUM") as psump,
            tc.tile_pool(name="dps", bufs=1, space="PSUM") as dpsp,
        ):
            xts = constp.tile([128, KT * B], bf16)
            prox = constp.tile([128, 2 * KT * W], fp8)
            pm = constp.tile([128, W], fp8)
            outt = constp.tile([128, ncols], f32)
            warm = constp.tile([128, 512], bf16)  # never written: garbage ok
            dps = dpsp.tile([128, 512], f32)      # dummy matmul sink

            # small loads on the Scalar HWDGE queue (parallel with the stream)
            nc.scalar.dma_start(out=xts[:], in_=xm_e[:, :])
            nc.scalar.dma_start(out=pm[:], in_=pm_e[:, :])

            # proxP stream on the Sync HWDGE queue, consumption order.
            # j0 quartered (fast ramp), middle tiles one 2MB chunk each (best
            # HBM rate without coarsening the PE chase), last two tiles
            # fine-grained for the end chase (final quarter split per half).
            poff = 0
            for j, w in enumerate(JTS):
                span = 2 * KT * w
                qspan = span // 4
                if j == 0:
                    for q in range(4):
                        lo = poff + q * qspan
                        nc.sync.dma_start(out=prox[:, lo:lo + qspan],
                                          in_=proxP_e[:, lo:lo + qspan])
                elif 1 <= j <= 6:
                    nc.sync.dma_start(out=prox[:, poff:poff + span],
                                      in_=proxP_e[:, poff:poff + span])
                elif j == NJT - 2:
                    for hlf in range(2):
                        lo = poff + hlf * (span // 2)
                        nc.sync.dma_start(out=prox[:, lo:lo + span // 2],
                                          in_=proxP_e[:, lo:lo + span // 2])
                else:
                    for q in range(3):
                        lo = poff + q * qspan
                        nc.sync.dma_start(out=prox[:, lo:lo + qspan],
                                          in_=proxP_e[:, lo:lo + qspan])
                    hq = qspan // 2
                    lo = poff + 3 * qspan
                    nc.sync.dma_start(out=prox[:, lo:lo + hq],
                                      in_=proxP_e[:, lo:lo + hq])
                    nc.sync.dma_start(out=prox[:, lo + hq:lo + qspan],
                                      in_=proxP_e[:, lo + hq:lo + qspan])
                poff += span

            nc.vector.memset(warm[:], 0.0)
            # HAM warmup: keep PE busy through the preamble/first-chunk window
            # so the clock gate opens before real matmuls start (garbage data,
            # dedicated PSUM bank, no dependencies).
            for _ in range(9):
                nc.tensor.matmul(dps[0:64, :], warm[:, 0:64], warm[:, :],
                                 start=True, stop=True)

            # matmul + DVE mask-add + top8 pieces per jtile
            poff = 0
            for j, w in enumerate(JTS):
                jo, _, pieces = lay[j]
                if 1 <= j <= 7:
                    # boundary filler: PE chews the previous tile's (already
                    # arrived) bytes while waiting for this tile's chunk — no
                    # >3.4us idle window, so HAM stays at full clock.
                    for f in range(3):
                        nc.tensor.matmul(dps[0:64, :], xts[:, 0:64],
                                         prox[:, poff - 512:poff],
                                         start=True, stop=True)
                ps = psump.tile([128, 512], f32, tag="ps")
                if j == NJT - 1:
                    # final quarter de-interleaved: all h0 mms before the h1
                    # mms so only h1's last eighth-chunk gates exposed work
                    korder = [(k, h) for k in range(12) for h in range(2)]
                    korder += [(k, 0) for k in range(12, 16)]
                    korder += [(k, 1) for k in range(12, 16)]
                else:
                    korder = [(k, h) for k in range(KT) for h in range(2)]
                for (k, h) in korder:
                    q, kk = divmod(k, 4)
                    xk = xts[:, k * B:(k + 1) * B]
                    bh = poff + ((q * 2 + h) * 4 + kk) * w
                    nc.tensor.matmul(ps[64 * h:64 * h + 64, :w], xk,
                                     prox[:, bh:bh + w],
                                     start=(k == 0), stop=(k == KT - 1))
                mn = scrp.tile([128, 512], f32, tag="mn")
                halves = ((slice(0, 64), slice(64, 128))
                          if j == NJT - 1 else (slice(0, 128),))
                for hs in halves:
                    nc.vector.scalar_tensor_tensor(
                        mn[hs, :w], pm[hs, jo:jo + w], -224.0, ps[hs, :w],
                        mybir.AluOpType.max, mybir.AluOpType.add)
                    for (c, h, lo, hi, off, wout) in pieces:
                        if 64 * h < hs.start or 64 * h >= hs.stop:
                            continue
                        pr = slice(64 * h, 64 * h + 64)
                        src = mn[pr, lo - jo:hi - jo]
                        if wout == 8:
                            nc.vector.max(outt[pr, off:off + 8], src)
                        else:
                            nc.vector.tensor_copy(out=outt[pr, off:off + wout], in_=src)
                if j == 6:
                    nc.scalar.dma_start(out=o_all[:, 0:bulk], in_=outt[:, 0:bulk])
                elif j == 7:
                    nc.scalar.dma_start(out=o_all[:, bulk:bulk2],
                                        in_=outt[:, bulk:bulk2])
                poff += 2 * KT * w

            nc.scalar.dma_start(out=o_all[:, bulk2:ncols], in_=outt[:, bulk2:ncols])

    nc.compile()
    return nc


# ---------------------------------------------------------------- host math
def _host_batch_terms(x, targets, cams, cids_hist, vals, D_cam):
    """Mirror of reference _acm_dis / _acm_ins with merged `vals`."""
    Bsz = x.shape[0]
    C = MAX_CAMS
    f32 = np.float32

    diff = x[:, None, :] - x[None, :, :]
    d2 = np.sum(diff * diff, axis=-1, dtype=f32)
    eye = np.eye(Bsz, dtype=bool)
    pw = np.sqrt(np.where(eye, f32(1.0), d2)).astype(f32) * (~eye)

    # ---- _acm_dis (MMD between intra/inter camera pair distances)
    iu, ju = np.triu_indices(Bsz, 1)
    dvec = pw[iu, ju].astype(f32)
    same = cams[iu] == cams[ju]
    wx = same.astype(f32)
    wy = (~same).astype(f32)
    n = wx.sum(dtype=f32)
    mm = wy.sum(dtype=f32)
    sq = (dvec[:, None] - dvec[None, :]) ** 2
    Sxx = wx @ sq @ wx
    Syy = wy @ sq @ wy
    denom = max(f32(1.0), n * n - n + mm * mm - mm)
    sigma = max(max(Sxx + Syy, f32(1e-6)) / denom, f32(1e-6))
    K = np.exp(-sq / sigma, dtype=f32)
    kxx = (wx @ K @ wx) / max(n * n, f32(1.0))
    kyy = (wy @ K @ wy) / max(mm * mm, f32(1.0))
    kxy = (wx @ K @ wy) / max(n * mm, f32(1.0))
    dis = (kxx + kyy - 2.0 * kxy) if (n >= 2 and mm >= 2) else f32(0.0)

    # ---- _acm_ins
    Moh = np.zeros((Bsz, C), dtype=f32)
    Moh[np.arange(Bsz), cams] = 1.0
    triu = np.triu(np.ones((Bsz, Bsz), dtype=f32), 1)
    pwt = pw * triu
    intra_sum = np.einsum('ic,jc,ij->c', Moh, Moh, pwt).astype(f32)
    intra_cnt = np.einsum('ic,jc,ij->c', Moh, Moh, triu).astype(f32)
    intra_mean = intra_sum / np.maximum(intra_cnt, 1.0)
    cam_cnt = Moh.sum(0)
    proxy_cnt = cids_hist.astype(f32)
    with np.errstate(invalid='ignore'):
        mean_d = (Moh.T @ vals) / np.maximum(cam_cnt, 1.0)[:, None]

    Dc = D_cam.astype(f32).copy()
    rng = np.arange(C)
    diag = np.diagonal(Dc).copy()
    diag_new = GAMMA * diag + (1.0 - GAMMA) * np.maximum(intra_mean, 1e-6)
    Dc[rng, rng] = np.where(intra_cnt >= 1.0, diag_new, diag)
    present = cam_cnt > 0
    off_mask = present[:, None] & present[None, :] & (proxy_cnt[None, :] > 0) & (~np.eye(C, dtype=bool))
    with np.errstate(invalid='ignore'):
        upd = GAMMA * Dc + (1.0 - GAMMA) * np.maximum(mean_d, 1e-6)
    Dc = np.where(off_mask, upd, Dc)
    Dc = np.maximum(Dc, 1e-6)

    dist_raw = np.maximum(1.0 - x @ x.T, 1e-8).astype(f32)
    pos_m = (targets[:, None] == targets[None, :]) & (cams[:, None] != cams[None, :])
    neg_m = targets[:, None] != targets[None, :]
    hard_pos = np.argmin(np.where(pos_m, dist_raw, np.inf), axis=1)
    hard_neg = np.argmax(np.where(neg_m, dist_raw, -np.inf), axis=1)
    idx = np.arange(Bsz)
    ic = cams
    scale_pos = np.clip(Dc[ic, ic] / Dc[ic, cams[hard_pos]], 0.1, 10.0)
    scale_neg = np.clip(Dc[ic, ic] / Dc[ic, cams[hard_neg]], 0.1, 10.0)
    d_pos = dist_raw[idx, hard_pos] * scale_pos
    d_neg = dist_raw[idx, hard_neg] * scale_neg
    trip = np.maximum(d_pos - d_neg + 0.2, 0.0)
    valid = (pos_m.sum(1) > 0) & (neg_m.sum(1) > 0)
    cnt = f32(valid.sum())
    ins = np.where(valid, trip, 0.0).sum(dtype=f32) / max(cnt, f32(1.0)) if cnt > 0 else f32(0.0)
    return f32(dis), f32(ins)


def _full_fallback(x, targets, cams, proxy, pids, cids, D_cam):
    """Exact numpy recompute of the main loss terms (guard-trip path)."""
    sims = (x @ proxy.T).astype(np.float32) / TEMP
    pos = (targets[:, None] == pids[None, :]) & (cams[:, None] != cids[None, :])
    neg = targets[:, None] != pids[None, :]
    scores = sims - 9999999.0 * (1.0 - neg.astype(np.float32))
    part = -np.partition(-scores, NUM_HARDS - 1, axis=1)[:, :NUM_HARDS]
    npos = pos.sum(1)
    mean_pos = np.where(npos > 0,
                        np.sum(np.where(pos, sims, 0.0), axis=1) / np.maximum(npos, 1), 0.0)
    with np.errstate(divide='ignore'):
        ps = np.where(pos, sims.astype(np.float64), -np.inf)
    pmax = ps.max(axis=1)
    lse_pos = np.where(np.isfinite(pmax),
                       pmax + np.log(np.sum(np.exp(ps - pmax[:, None]), axis=1)), -np.inf)
    nmax = part.max(axis=1)
    lse_neg = nmax + np.log(np.sum(np.exp(part - nmax[:, None]), axis=1))
    lse = np.logaddexp(lse_pos, lse_neg)
    row = np.where(npos > 0, lse - mean_pos, 0.0)
    loss = row.sum() / B
    vals = np.full((B, MAX_CAMS), np.inf, dtype=np.float32)
    dall = np.maximum(1.0 - x @ proxy.T, 1e-8).astype(np.float32)
    for c in range(NCAMS):
        cols = cids == c
        if cols.sum() >= 1:
            sub = dall[:, cols]
            k = min(NK, sub.shape[1])
            vals[:, c] = np.partition(sub, k - 1, axis=1)[:, :k].mean(axis=1)
    return loss, vals


# ---------------------------------------------------------------- entry point
def kernel(**inputs):
    inp = np.asarray(inputs["inputs"], dtype=np.float32)
    targets = np.asarray(inputs["targets"]).astype(np.int64)
    cams = np.asarray(inputs["cams"]).astype(np.int64)
    proxy = np.asarray(inputs["proxy"], dtype=np.float32)
    pids = np.asarray(inputs["pids"]).astype(np.int64)
    cids = np.asarray(inputs["cids"]).astype(np.int64)
    D_cam = np.asarray(inputs["D_cam"], dtype=np.float32)

    x = inp / np.maximum(np.linalg.norm(inp, axis=1, keepdims=True), 1e-12)
    x = x.astype(np.float32)
    xP = x.T.reshape(KT, 128, B).transpose(1, 0, 2).reshape(
        128, KT * B).astype(ml_dtypes.bfloat16)

    idx_by_cam, dealt, slot, offs = _plan(cids)
    lay, ncols, _ = _layout(slot, offs)

    key = tuple(sorted(slot.items()))
    if key not in _cache:
        _cache[key] = _build(slot, offs)
    nc = _cache[key]

    in_maps = []
    for m in range(M):
        im = _prep_core(m, dealt, slot, offs, proxy, targets, pids)
        im["xm"] = xP
        in_maps.append(im)

    res = run_bass_kernel_spmd(nc, in_maps, core_ids=list(range(M)))
    outs = res.results

    # ------------- host-exact side pools -------------
    # (a) class-matching columns, grouped by distinct target
    a_pos = [None] * B        # per row: sims of its positives (pid==t, cid!=cam)
    a_cam = [[None] * NCAMS for _ in range(B)]  # per row, cam: sims of pid==t cols
    for t in np.unique(targets):
        rows_t = np.nonzero(targets == t)[0]
        cols_t = np.nonzero(pids == t)[0]
        if len(cols_t) == 0:
            for i in rows_t:
                a_pos[i] = np.zeros(0, dtype=np.float32)
                for c in range(NCAMS):
                    a_cam[i][c] = np.zeros(0, dtype=np.float32)
            continue
        S = (x[rows_t] @ proxy[cols_t].T).astype(np.float32)
        cid_t = cids[cols_t]
        for r, i in enumerate(rows_t):
            a_pos[i] = S[r][cid_t != cams[i]]
            for c in range(NCAMS):
                a_cam[i][c] = S[r][cid_t == c]

    # (b) overflow columns (dealt beyond the global slot), all rows
    of_cols = np.concatenate(
        [dealt[m][c][slot[c]:] for m in range(M) for c in range(NCAMS)]
        + [np.zeros(0, dtype=np.int64)])
    if len(of_cols):
        S_b = (x @ proxy[of_cols].T).astype(np.float32)
        pid_b = pids[of_cols]
        cid_b = cids[of_cols]
    else:
        S_b = np.zeros((B, 0), dtype=np.float32)
        pid_b = np.zeros(0, dtype=np.int64)
        cid_b = np.zeros(0, dtype=np.int64)

    # ------------- parse device outputs -------------
    # guarded pieces (top8 of >8 cols) and raw pieces, per cam
    guard_chunks = []          # list of (64, 8) device top8 arrays
    cam_dev = [[] for _ in range(NCAMS)]
    for m in range(M):
        oa = np.asarray(outs[m]["o_all"], dtype=np.float32)
        for (jo, w, pieces) in lay:
            for (c, h, lo, hi, off, wout) in pieces:
                v = oa[64 * h:64 * h + 64, off:off + wout]
                cam_dev[c].append((v, hi - lo))
                if wout == 8 and hi - lo > 8:
                    guard_chunks.append(v)

    all_dev = np.concatenate([v for c in range(NCAMS) for (v, _) in cam_dev[c]],
                             axis=1)  # (64, P) masked d values, negatives only

    # ------------- hard-negative top-50 + logsumexp -------------
    fallback = False
    lse_neg = np.empty(B)
    for i in range(B):
        pool = np.concatenate([all_dev[i], S_b[i][pid_b != targets[i]]])
        top50 = -np.partition(-pool, NUM_HARDS - 1)[:NUM_HARDS]
        thr = top50[-1]
        t = top50.astype(np.float64) / TEMP
        tm = t.max()
        lse_neg[i] = tm + np.log(np.exp(t - tm).sum())
        for g in guard_chunks:
            if (g[i] >= thr).sum() >= 8:
                fallback = True
        if fallback:
            break

    cids_hist = np.zeros(MAX_CAMS)
    np.add.at(cids_hist, cids, 1.0)
    present = np.zeros(MAX_CAMS)
    np.add.at(present, cams, 1.0)
    multi_cam = (present > 0).sum() >= 2

    if fallback:
        loss, vals = _full_fallback(x, targets, cams, proxy, pids, cids, D_cam)
    else:
        # positives: exact f32
        lse_pos = np.full(B, -np.inf)
        mean_pos = np.zeros(B)
        npos = np.zeros(B, dtype=np.int64)
        for i in range(B):
            v = a_pos[i].astype(np.float64) / TEMP
            npos[i] = len(v)
            if len(v):
                tm = v.max()
                lse_pos[i] = tm + np.log(np.exp(v - tm).sum())
                mean_pos[i] = v.mean()
        lse = np.logaddexp(lse_pos, lse_neg)
        row = np.where(npos > 0, lse - mean_pos, 0.0)
        loss = row.sum() / B

        # per-(row, cam) top-5 nearest proxies
        vals = np.full((B, MAX_CAMS), np.inf, dtype=np.float32)
        for c in range(NCAMS):
            dev_c = np.concatenate([v for (v, _) in cam_dev[c]], axis=1)
            b_c = (pid_b != -1) & (cid_b == c)
            for i in range(B):
                cand = np.concatenate([
                    dev_c[i], a_cam[i][c],
                    S_b[i][b_c & (pid_b != targets[i])]])
                k = min(NK, len(cand))
                if k:
                    topk = -np.partition(-cand, k - 1)[:k]
                    vals[i, c] = np.maximum(1.0 - topk, 1e-8).mean()

    if multi_cam:
        dis, ins = _host_batch_terms(x, targets, cams, cids_hist, vals, D_cam)
        loss = loss + LAM_DIS * float(dis) + LAM_INS * float(ins)

    return np.float32(loss)


# revision 20
# speedup vs baseline: 1.1581x; 1.1581x over previous
"""Distributed Trainium2 Bass kernel for the ACMProxy loss.

Sharding: proxy bank (N=65536) split across 8 NeuronCores, camera-grouped and
evenly dealt so all cores share one SPMD graph. Each core runs the heavy
(B=64)x(8192) sims matmul (bf16 x stationary, fp8 proxies moving, f32 PSUM)
with the per-row class-match mask folded into PSUM via a 17th accumulation
matmul (identity weights x fp8 mask plane). The only epilogue is a top-8
(InstMax) per (camera x jtile) piece, read straight from PSUM.

Host does everything exact and tiny: positives / class-matching columns
(~7.6k of 65536) and the few overflow proxies are rescored in f32 numpy;
device piece top-8s are merged for hard-negative top-50 (with a per-piece
saturation guard + full-numpy fallback) and per-camera top-5; batch-level
MMD/triplet terms are computed in numpy.

Layout: per (core, half) exactly W=4096 proxy columns (3 cams contiguous,
global slot sizes), jtiles 7x512 + 2x256. DMA: proxP repacked so each
(jtile, k-quarter) is one contiguous 0.5MB chunk in consumption order
(k-major, halves interleaved per quarter); issued on the Sync HWDGE queue.
xm/mask/identity + output flushes ride the Scalar HWDGE queue. The final
quarter is split per-half so almost no matmul work is exposed after the
last streamed byte.
"""

import ml_dtypes
import numpy as np

import concourse.mybir as mybir
from concourse import bacc
from concourse.tile import TileContext
from concourse.bass_utils import run_bass_kernel_spmd

# problem constants (hardcoded)
B, D, N = 64, 2048, 65536
M = 8
KT = D // 128
TEMP = 0.07
NUM_HARDS = 50
LAM_DIS = 0.05
LAM_INS = 0.05
GAMMA = 0.9
NK = 5
MAX_CAMS = 8
NCAMS = 6

W = 4096                      # columns per (core, half)
JTS = [512] * 7 + [256, 256]  # jtile widths (sum == W)
NJT = len(JTS)
HALVES = ((0, 1, 2), (3, 4, 5))
MASK = -224.0                 # fp8e4m3-exact exclusion value; real d in [-6, 6]

_cache = {}


# ---------------------------------------------------------------- layout plan
def _plan(cids):
    """Global cam slots (same for every core) + per-core dealt indices."""
    idx_by_cam = [np.nonzero(cids == c)[0] for c in range(NCAMS)]
    dealt = [[idx_by_cam[c][m::M] for c in range(NCAMS)] for m in range(M)]
    slot = {}
    for camlist in HALVES:
        base = [len(idx_by_cam[c]) // M for c in camlist]
        total = sum(base)
        i = 0
        while total < W:
            base[i % 3] += 1
            total += 1
            i += 1
        while total > W:
            j = int(np.argmax(base))
            base[j] -= 1
            total -= 1
        for c, s in zip(camlist, base):
            slot[c] = s
    offs = {}
    for camlist in HALVES:
        o = 0
        for c in camlist:
            offs[c] = o
            o += slot[c]
    return idx_by_cam, dealt, slot, offs


def _layout(slot, offs):
    """Per-jtile pieces: (cam, half, lo, hi, out_col, wout). wout=8 -> top8,
    wout<8 -> raw copy of the (narrow) intersection."""
    lay = []
    col = 0
    jo = 0
    jstart = []
    for w in JTS:
        jstart.append(col)
        pieces = []
        for h in range(2):
            for c in HALVES[h]:
                lo = max(offs[c], jo)
                hi = min(offs[c] + slot[c], jo + w)
                if lo < hi:
                    wout = 8 if hi - lo >= 8 else hi - lo
                    pieces.append((c, h, lo, hi, col, wout))
                    col += wout
        lay.append((jo, w, pieces))
        jo += w
    # flush [0, bulk) after j6's pieces, [bulk, bulk2) after j7's, rest at end
    return lay, col, (jstart[7], jstart[8])


def _prep_core(m, dealt, slot, offs, proxy, targets, pids):
    """Per-core device inputs: repacked fp8 proxies + fp8 mask plane."""
    col_g = np.full((2, W), -1, dtype=np.int64)
    for c in range(NCAMS):
        h = 0 if c < 3 else 1
        g = dealt[m][c][:slot[c]]
        col_g[h, offs[c]:offs[c] + len(g)] = g
    real = col_g >= 0

    proxT = np.zeros((D, 2, W), dtype=np.float32)
    proxT[:, real] = proxy[col_g[real], :].T

    pid_col = np.where(real, pids[np.where(real, col_g, 0)], -1)  # (2, W)
    # mask plane rows 64h+i: 0 where the column is a negative for batch row i
    # (valid and class mismatch), MASK elsewhere (pad or class match).
    plane = np.empty((128, W), dtype=np.float32)
    for h in range(2):
        neg = real[h][None, :] & (targets[:, None] != pid_col[h][None, :])
        plane[64 * h:64 * h + 64] = np.where(neg, 0.0, MASK)

    # repack fp8, per jtile: [q(4)][h(2)][kk(4)][w] (consumption order)
    A = proxT.astype(ml_dtypes.float8_e4m3).reshape(KT, 128, 2, W)
    parts = []
    jo = 0
    for w in JTS:
        blk = A[:, :, :, jo:jo + w]                # (KT,128,2,w)
        b = blk.reshape(4, 4, 128, 2, w)           # (q,kk,128,h,w)
        b = np.transpose(b, (2, 0, 3, 1, 4))       # (128,q,h,kk,w)
        parts.append(b.reshape(128, 2 * KT * w))
        jo += w
    proxP = np.ascontiguousarray(np.concatenate(parts, axis=1))
    return {
        "proxP": proxP,
        "pm": np.ascontiguousarray(plane.astype(ml_dtypes.float8_e4m3)),
    }


# ---------------------------------------------------------------- bass kernel
def _build(slot, offs):
    f32 = mybir.dt.float32
    bf16 = mybir.dt.bfloat16
    fp8 = mybir.dt.float8e4
    lay, ncols, (bulk, bulk2) = _layout(slot, offs)
    nc = bacc.Bacc("TRN2", target_bir_lowering=False, debug=False, num_devices=M)

    proxP_e = nc.dram_tensor("proxP", [128, 2 * KT * W], fp8, kind="ExternalInput").ap()
    xm_e = nc.dram_tensor("xm", [128, KT * B], bf16, kind="ExternalInput").ap()
    pm_e = nc.dram_tensor("pm", [128, W], fp8, kind="ExternalInput").ap()
    o_all = nc.dram_tensor("o_all", [128, ncols], f32, kind="ExternalOutput").ap()

    with TileContext(nc) as tc:
        with (
            tc.tile_pool(name="const", bufs=1) as constp,
            tc.tile_pool(name="scr", bufs=3) as scrp,
            tc.tile_pool(name="ps", bufs=8, space="PSUM") as psump,
        ):
            xts = constp.tile([128, KT * B], bf16)
            prox = constp.tile([128, 2 * KT * W], fp8)
            pm = constp.tile([128, W], fp8)
            outt = constp.tile([128, ncols], f32)

            # small loads on the Scalar HWDGE queue (parallel with the stream)
            nc.scalar.dma_start(out=xts[:], in_=xm_e[:, :])
            nc.scalar.dma_start(out=pm[:], in_=pm_e[:, :])

            # proxP stream on the Sync HWDGE queue, consumption order.
            # j0 quartered (fast ramp), middle tiles one 2MB chunk each (best
            # HBM rate without coarsening the PE chase), last two tiles
            # fine-grained for the end chase (final quarter split per half).
            poff = 0
            for j, w in enumerate(JTS):
                span = 2 * KT * w
                qspan = span // 4
                if j == 0:
                    for q in range(4):
                        lo = poff + q * qspan
                        nc.sync.dma_start(out=prox[:, lo:lo + qspan],
                                          in_=proxP_e[:, lo:lo + qspan])
                elif j in (1, 2):
                    # half-chunks: j0 was quartered so the PE gets here early;
                    # a full-tile wait would be a >3.4us PE idle (HAM cold)
                    for hlf in range(2):
                        lo = poff + hlf * (span // 2)
                        nc.sync.dma_start(out=prox[:, lo:lo + span // 2],
                                          in_=proxP_e[:, lo:lo + span // 2])
                elif 3 <= j <= 6:
                    nc.sync.dma_start(out=prox[:, poff:poff + span],
                                      in_=proxP_e[:, poff:poff + span])
                elif j == NJT - 2:
                    for hlf in range(2):
                        lo = poff + hlf * (span // 2)
                        nc.sync.dma_start(out=prox[:, lo:lo + span // 2],
                                          in_=proxP_e[:, lo:lo + span // 2])
                else:
                    for q in range(3):
                        lo = poff + q * qspan
                        nc.sync.dma_start(out=prox[:, lo:lo + qspan],
                                          in_=proxP_e[:, lo:lo + qspan])
                    hq = qspan // 2
                    lo = poff + 3 * qspan
                    nc.sync.dma_start(out=prox[:, lo:lo + hq],
                                      in_=proxP_e[:, lo:lo + hq])
                    nc.sync.dma_start(out=prox[:, lo + hq:lo + qspan],
                                      in_=proxP_e[:, lo + hq:lo + qspan])
                poff += span

            # matmul + DVE mask-add + top8 pieces per jtile
            poff = 0
            for j, w in enumerate(JTS):
                jo, _, pieces = lay[j]
                ps = psump.tile([128, 512], f32, tag="ps")
                if j == NJT - 1:
                    # final quarter de-interleaved: all h0 mms before the h1
                    # mms so only h1's last eighth-chunk gates exposed work
                    korder = [(k, h) for k in range(12) for h in range(2)]
                    korder += [(k, 0) for k in range(12, 16)]
                    korder += [(k, 1) for k in range(12, 16)]
                else:
                    korder = [(k, h) for k in range(KT) for h in range(2)]
                for (k, h) in korder:
                    q, kk = divmod(k, 4)
                    xk = xts[:, k * B:(k + 1) * B]
                    bh = poff + ((q * 2 + h) * 4 + kk) * w
                    nc.tensor.matmul(ps[64 * h:64 * h + 64, :w], xk,
                                     prox[:, bh:bh + w],
                                     start=(k == 0), stop=(k == KT - 1))
                mn = scrp.tile([128, 512], f32, tag="mn")
                halves = ((slice(0, 64), slice(64, 128))
                          if j == NJT - 1 else (slice(0, 128),))
                for hs in halves:
                    nc.vector.scalar_tensor_tensor(
                        mn[hs, :w], pm[hs, jo:jo + w], -224.0, ps[hs, :w],
                        mybir.AluOpType.max, mybir.AluOpType.add)
                    for (c, h, lo, hi, off, wout) in pieces:
                        if 64 * h < hs.start or 64 * h >= hs.stop:
                            continue
                        pr = slice(64 * h, 64 * h + 64)
                        src = mn[pr, lo - jo:hi - jo]
                        if wout == 8:
                            nc.vector.max(outt[pr, off:off + 8], src)
                        else:
                            nc.vector.tensor_copy(out=outt[pr, off:off + wout], in_=src)
                if j == 6:
                    nc.scalar.dma_start(out=o_all[:, 0:bulk], in_=outt[:, 0:bulk])
                elif j == 7:
                    nc.scalar.dma_start(out=o_all[:, bulk:bulk2],
                                        in_=outt[:, bulk:bulk2])
                poff += 2 * KT * w

            nc.scalar.dma_start(out=o_all[:, bulk2:ncols], in_=outt[:, bulk2:ncols])

    nc.compile()
    return nc


# ---------------------------------------------------------------- host math
def _host_batch_terms(x, targets, cams, cids_hist, vals, D_cam):
    """Mirror of reference _acm_dis / _acm_ins with merged `vals`."""
    Bsz = x.shape[0]
    C = MAX_CAMS
    f32 = np.float32

    diff = x[:, None, :] - x[None, :, :]
    d2 = np.sum(diff * diff, axis=-1, dtype=f32)
    eye = np.eye(Bsz, dtype=bool)
    pw = np.sqrt(np.where(eye, f32(1.0), d2)).astype(f32) * (~eye)

    # ---- _acm_dis (MMD between intra/inter camera pair distances)
    iu, ju = np.triu_indices(Bsz, 1)
    dvec = pw[iu, ju].astype(f32)
    same = cams[iu] == cams[ju]
    wx = same.astype(f32)
    wy = (~same).astype(f32)
    n = wx.sum(dtype=f32)
    mm = wy.sum(dtype=f32)
    sq = (dvec[:, None] - dvec[None, :]) ** 2
    Sxx = wx @ sq @ wx
    Syy = wy @ sq @ wy
    denom = max(f32(1.0), n * n - n + mm * mm - mm)
    sigma = max(max(Sxx + Syy, f32(1e-6)) / denom, f32(1e-6))
    K = np.exp(-sq / sigma, dtype=f32)
    kxx = (wx @ K @ wx) / max(n * n, f32(1.0))
    kyy = (wy @ K @ wy) / max(mm * mm, f32(1.0))
    kxy = (wx @ K @ wy) / max(n * mm, f32(1.0))
    dis = (kxx + kyy - 2.0 * kxy) if (n >= 2 and mm >= 2) else f32(0.0)

    # ---- _acm_ins
    Moh = np.zeros((Bsz, C), dtype=f32)
    Moh[np.arange(Bsz), cams] = 1.0
    triu = np.triu(np.ones((Bsz, Bsz), dtype=f32), 1)
    pwt = pw * triu
    intra_sum = np.einsum('ic,jc,ij->c', Moh, Moh, pwt).astype(f32)
    intra_cnt = np.einsum('ic,jc,ij->c', Moh, Moh, triu).astype(f32)
    intra_mean = intra_sum / np.maximum(intra_cnt, 1.0)
    cam_cnt = Moh.sum(0)
    proxy_cnt = cids_hist.astype(f32)
    with np.errstate(invalid='ignore'):
        mean_d = (Moh.T @ vals) / np.maximum(cam_cnt, 1.0)[:, None]

    Dc = D_cam.astype(f32).copy()
    rng = np.arange(C)
    diag = np.diagonal(Dc).copy()
    diag_new = GAMMA * diag + (1.0 - GAMMA) * np.maximum(intra_mean, 1e-6)
    Dc[rng, rng] = np.where(intra_cnt >= 1.0, diag_new, diag)
    present = cam_cnt > 0
    off_mask = present[:, None] & present[None, :] & (proxy_cnt[None, :] > 0) & (~np.eye(C, dtype=bool))
    with np.errstate(invalid='ignore'):
        upd = GAMMA * Dc + (1.0 - GAMMA) * np.maximum(mean_d, 1e-6)
    Dc = np.where(off_mask, upd, Dc)
    Dc = np.maximum(Dc, 1e-6)

    dist_raw = np.maximum(1.0 - x @ x.T, 1e-8).astype(f32)
    pos_m = (targets[:, None] == targets[None, :]) & (cams[:, None] != cams[None, :])
    neg_m = targets[:, None] != targets[None, :]
    hard_pos = np.argmin(np.where(pos_m, dist_raw, np.inf), axis=1)
    hard_neg = np.argmax(np.where(neg_m, dist_raw, -np.inf), axis=1)
    idx = np.arange(Bsz)
    ic = cams
    scale_pos = np.clip(Dc[ic, ic] / Dc[ic, cams[hard_pos]], 0.1, 10.0)
    scale_neg = np.clip(Dc[ic, ic] / Dc[ic, cams[hard_neg]], 0.1, 10.0)
    d_pos = dist_raw[idx, hard_pos] * scale_pos
    d_neg = dist_raw[idx, hard_neg] * scale_neg
    trip = np.maximum(d_pos - d_neg + 0.2, 0.0)
    valid = (pos_m.sum(1) > 0) & (neg_m.sum(1) > 0)
    cnt = f32(valid.sum())
    ins = np.where(valid, trip, 0.0).sum(dtype=f32) / max(cnt, f32(1.0)) if cnt > 0 else f32(0.0)
    return f32(dis), f32(ins)


def _full_fallback(x, targets, cams, proxy, pids, cids, D_cam):
    """Exact numpy recompute of the main loss terms (guard-trip path)."""
    sims = (x @ proxy.T).astype(np.float32) / TEMP
    pos = (targets[:, None] == pids[None, :]) & (cams[:, None] != cids[None, :])
    neg = targets[:, None] != pids[None, :]
    scores = sims - 9999999.0 * (1.0 - neg.astype(np.float32))
    part = -np.partition(-scores, NUM_HARDS - 1, axis=1)[:, :NUM_HARDS]
    npos = pos.sum(1)
    mean_pos = np.where(npos > 0,
                        np.sum(np.where(pos, sims, 0.0), axis=1) / np.maximum(npos, 1), 0.0)
    with np.errstate(divide='ignore'):
        ps = np.where(pos, sims.astype(np.float64), -np.inf)
    pmax = ps.max(axis=1)
    lse_pos = np.where(np.isfinite(pmax),
                       pmax + np.log(np.sum(np.exp(ps - pmax[:, None]), axis=1)), -np.inf)
    nmax = part.max(axis=1)
    lse_neg = nmax + np.log(np.sum(np.exp(part - nmax[:, None]), axis=1))
    lse = np.logaddexp(lse_pos, lse_neg)
    row = np.where(npos > 0, lse - mean_pos, 0.0)
    loss = row.sum() / B
    vals = np.full((B, MAX_CAMS), np.inf, dtype=np.float32)
    dall = np.maximum(1.0 - x @ proxy.T, 1e-8).astype(np.float32)
    for c in range(NCAMS):
        cols = cids == c
        if cols.sum() >= 1:
            sub = dall[:, cols]
            k = min(NK, sub.shape[1])
            vals[:, c] = np.partition(sub, k - 1, axis=1)[:, :k].mean(axis=1)
    return loss, vals


# ---------------------------------------------------------------- entry point
def kernel(**inputs):
    inp = np.asarray(inputs["inputs"], dtype=np.float32)
    targets = np.asarray(inputs["targets"]).astype(np.int64)
    cams = np.asarray(inputs["cams"]).astype(np.int64)
    proxy = np.asarray(inputs["proxy"], dtype=np.float32)
    pids = np.asarray(inputs["pids"]).astype(np.int64)
    cids = np.asarray(inputs["cids"]).astype(np.int64)
    D_cam = np.asarray(inputs["D_cam"], dtype=np.float32)

    x = inp / np.maximum(np.linalg.norm(inp, axis=1, keepdims=True), 1e-12)
    x = x.astype(np.float32)
    xP = x.T.reshape(KT, 128, B).transpose(1, 0, 2).reshape(
        128, KT * B).astype(ml_dtypes.bfloat16)

    idx_by_cam, dealt, slot, offs = _plan(cids)
    lay, ncols, _ = _layout(slot, offs)

    key = tuple(sorted(slot.items()))
    if key not in _cache:
        _cache[key] = _build(slot, offs)
    nc = _cache[key]

    in_maps = []
    for m in range(M):
        im = _prep_core(m, dealt, slot, offs, proxy, targets, pids)
        im["xm"] = xP
        in_maps.append(im)

    res = run_bass_kernel_spmd(nc, in_maps, core_ids=list(range(M)))
    outs = res.results

    # ------------- host-exact side pools -------------
    # (a) class-matching columns, grouped by distinct target
    a_pos = [None] * B        # per row: sims of its positives (pid==t, cid!=cam)
    a_cam = [[None] * NCAMS for _ in range(B)]  # per row, cam: sims of pid==t cols
    for t in np.unique(targets):
        rows_t = np.nonzero(targets == t)[0]
        cols_t = np.nonzero(pids == t)[0]
        if len(cols_t) == 0:
            for i in rows_t:
                a_pos[i] = np.zeros(0, dtype=np.float32)
                for c in range(NCAMS):
                    a_cam[i][c] = np.zeros(0, dtype=np.float32)
            continue
        S = (x[rows_t] @ proxy[cols_t].T).astype(np.float32)
        cid_t = cids[cols_t]
        for r, i in enumerate(rows_t):
            a_pos[i] = S[r][cid_t != cams[i]]
            for c in range(NCAMS):
                a_cam[i][c] = S[r][cid_t == c]

    # (b) overflow columns (dealt beyond the global slot), all rows
    of_cols = np.concatenate(
        [dealt[m][c][slot[c]:] for m in range(M) for c in range(NCAMS)]
        + [np.zeros(0, dtype=np.int64)])
    if len(of_cols):
        S_b = (x @ proxy[of_cols].T).astype(np.float32)
        pid_b = pids[of_cols]
        cid_b = cids[of_cols]
    else:
        S_b = np.zeros((B, 0), dtype=np.float32)
        pid_b = np.zeros(0, dtype=np.int64)
        cid_b = np.zeros(0, dtype=np.int64)

    # ------------- parse device outputs -------------
    # guarded pieces (top8 of >8 cols) and raw pieces, per cam
    guard_chunks = []          # list of (64, 8) device top8 arrays
    cam_dev = [[] for _ in range(NCAMS)]
    for m in range(M):
        oa = np.asarray(outs[m]["o_all"], dtype=np.float32)
        for (jo, w, pieces) in lay:
            for (c, h, lo, hi, off, wout) in pieces:
                v = oa[64 * h:64 * h + 64, off:off + wout]
                cam_dev[c].append((v, hi - lo))
                if wout == 8 and hi - lo > 8:
                    guard_chunks.append(v)

    all_dev = np.concatenate([v for c in range(NCAMS) for (v, _) in cam_dev[c]],
                             axis=1)  # (64, P) masked d values, negatives only

    # ------------- hard-negative top-50 + logsumexp -------------
    fallback = False
    lse_neg = np.empty(B)
    for i in range(B):
        pool = np.concatenate([all_dev[i], S_b[i][pid_b != targets[i]]])
        top50 = -np.partition(-pool, NUM_HARDS - 1)[:NUM_HARDS]
        thr = top50[-1]
        t = top50.astype(np.float64) / TEMP
        tm = t.max()
        lse_neg[i] = tm + np.log(np.exp(t - tm).sum())
        for g in guard_chunks:
            if (g[i] >= thr).sum() >= 8:
                fallback = True
        if fallback:
            break

    cids_hist = np.zeros(MAX_CAMS)
    np.add.at(cids_hist, cids, 1.0)
    present = np.zeros(MAX_CAMS)
    np.add.at(present, cams, 1.0)
    multi_cam = (present > 0).sum() >= 2

    if fallback:
        loss, vals = _full_fallback(x, targets, cams, proxy, pids, cids, D_cam)
    else:
        # positives: exact f32
        lse_pos = np.full(B, -np.inf)
        mean_pos = np.zeros(B)
        npos = np.zeros(B, dtype=np.int64)
        for i in range(B):
            v = a_pos[i].astype(np.float64) / TEMP
            npos[i] = len(v)
            if len(v):
                tm = v.max()
                lse_pos[i] = tm + np.log(np.exp(v - tm).sum())
                mean_pos[i] = v.mean()
        lse = np.logaddexp(lse_pos, lse_neg)
        row = np.where(npos > 0, lse - mean_pos, 0.0)
        loss = row.sum() / B

        # per-(row, cam) top-5 nearest proxies
        vals = np.full((B, MAX_CAMS), np.inf, dtype=np.float32)
        for c in range(NCAMS):
            dev_c = np.concatenate([v for (v, _) in cam_dev[c]], axis=1)
            b_c = (pid_b != -1) & (cid_b == c)
            for i in range(B):
                cand = np.concatenate([
                    dev_c[i], a_cam[i][c],
                    S_b[i][b_c & (pid_b != targets[i])]])
                k = min(NK, len(cand))
                if k:
                    topk = -np.partition(-cand, k - 1)[:k]
                    vals[i, c] = np.maximum(1.0 - topk, 1e-8).mean()

    if multi_cam:
        dis, ins = _host_batch_terms(x, targets, cams, cids_hist, vals, D_cam)
        loss = loss + LAM_DIS * float(dis) + LAM_INS * float(ins)

    return np.float32(loss)
